# revision 5
# baseline (speedup 1.0000x reference)
"""Differential attention kernel for 8 trn2 NeuronCores.

Sharding: (batch, head-group) over 8 cores. Core d handles batch b=d//4 and
heads [4*(d%4), 4*(d%4)+4). Each core:
  - projects q1,q2,k1,k2 in transposed layout qkT [1024, S] (channel on
    partitions) and v in direct layout [S, 4, 65] (ones column appended),
  - computes scores TRANSPOSED sT[j, i] (keys on partitions) so the attention
    mask bias is a per-partition ACT bias and the PV matmul needs no
    transposes,
  - exp via ACT (softmax scale folded into the activation), denominators fall
    out of the ones-augmented V column as row 64 of uT,
  - softmax normalization applied to the tiny uT [65, 512] outputs at the
    end: o_h = u1/dn1 - lam*u2/dn2 (+ (1-lam)*bv correction),
  - out-projection Wo_slice.T @ oT -> partial outT [1024, S].
Host sums the 4 partial outT per batch (+bo) and transposes.
"""
import numpy as np

B, S, D, H = 2, 2048, 1024, 16
DH = D // H          # 64
SCALE = DH ** -0.5   # 0.125
NCORES = 8
HG = 4               # heads per device
KT = D // 128        # 8 contraction tiles over D
MT = D // 128        # 8 output tiles of qk projection (q1,q2,k1,k2 cols)
NCH = S // 512       # 4 query chunks
JT = S // 128        # 16 key tiles

_BUILD_CACHE = {}


def _build(lam: float):
    from contextlib import ExitStack
    import concourse.mybir as mybir
    import concourse.tile as tile
    from concourse import bacc

    f32 = mybir.dt.float32
    f32r = mybir.dt.float32r
    Exp = mybir.ActivationFunctionType.Exp
    mult = mybir.AluOpType.mult
    add = mybir.AluOpType.add
    subtract = mybir.AluOpType.subtract

    nc = bacc.Bacc("TRN2", target_bir_lowering=False, debug=False,
                   num_devices=NCORES)

    xt_d = nc.dram_tensor("xt", [D, S], f32r, kind="ExternalInput").ap()
    wqk_d = nc.dram_tensor("wqk", [D, D], f32r, kind="ExternalInput").ap()
    wv_d = nc.dram_tensor("wv", [D, HG * DH], f32r, kind="ExternalInput").ap()
    wo_d = nc.dram_tensor("wo", [HG * DH, D], f32r, kind="ExternalInput").ap()
    bqk_d = nc.dram_tensor("bqk", [128, MT], f32, kind="ExternalInput").ap()
    bvc_d = nc.dram_tensor("bvc", [64, HG], f32, kind="ExternalInput").ap()
    maskb_d = nc.dram_tensor("maskb", [128, JT], f32,
                             kind="ExternalInput").ap()
    out_d = nc.dram_tensor("outT", [D, S], f32, kind="ExternalOutput").ap()

    with tile.TileContext(nc) as tc, ExitStack() as ctx:
        consts = ctx.enter_context(tc.tile_pool(name="consts", bufs=1))
        qk_pool = ctx.enter_context(tc.tile_pool(name="qk", bufs=1))
        v_pool = ctx.enter_context(tc.tile_pool(name="vp", bufs=1))
        ps = ctx.enter_context(tc.tile_pool(name="ps", bufs=1, space="PSUM"))

        bqk_sb = consts.tile([128, MT], f32)
        nc.sync.dma_start(out=bqk_sb, in_=bqk_d)
        bvc_sb = consts.tile([64, HG], f32)
        nc.sync.dma_start(out=bvc_sb, in_=bvc_d)
        maskb_sb = consts.tile([128, JT], f32)
        nc.sync.dma_start(out=maskb_sb, in_=maskb_d)
        # Wo rows grouped per local head hl: wo_sb[hl] = Wo_s[hl*64:(hl+1)*64]
        wo_sb = [consts.tile([64, D], f32r, name=f"wo{i}", tag=f"wo{i}")
                 for i in range(HG)]
        for i in range(HG):
            nc.sync.dma_start(out=wo_sb[i], in_=wo_d[i * 64:(i + 1) * 64, :])

        # v in [S, HG, DH+1] layout; column DH holds ones (denominator trick)
        v_sb = v_pool.tile([128, JT, HG, DH + 1], f32r)
        ones1 = consts.tile([128, 1], f32)
        nc.vector.memset(ones1, 1.0)
        # fp32r memset is not a valid ISA op; copy-broadcast ones instead
        nc.vector.tensor_copy(
            out=v_sb[:, :, :, DH:DH + 1],
            in_=ones1[:, None, None, :].broadcast_to([128, JT, HG, 1]))

        # qkT tiles: mt 0..7 = [q1 h01, q1 h23, q2 h01, q2 h23,
        #                       k1 h01, k1 h23, k2 h01, k2 h23]
        qkt = [qk_pool.tile([128, S], f32r, name=f"qkt{i}", tag=f"qkt{i}")
               for i in range(MT)]

        # ---------------- projections ----------------
        projstack = ExitStack()
        projw = projstack.enter_context(tc.tile_pool(name="projw", bufs=1))
        projx = projstack.enter_context(tc.tile_pool(name="projx", bufs=1))

        wqk_sb = [projw.tile([128, D], f32r, name=f"wqk{k}", tag=f"wqk{k}")
                  for k in range(KT)]
        for k in range(KT):
            nc.sync.dma_start(out=wqk_sb[k],
                              in_=wqk_d[k * 128:(k + 1) * 128, :])
        wv_sb = [projw.tile([128, HG * DH], f32r, name=f"wv{k}", tag=f"wv{k}")
                 for k in range(KT)]
        for k in range(KT):
            nc.sync.dma_start(out=wv_sb[k],
                              in_=wv_d[k * 128:(k + 1) * 128, :])

        for nc_i in range(NCH):
            nsl = slice(nc_i * 512, (nc_i + 1) * 512)
            xtc = []
            for k in range(KT):
                x_one = projx.tile([128, 512], f32r, name="xtc", tag="xtc",
                                   bufs=12)
                nc.sync.dma_start(out=x_one,
                                  in_=xt_d[k * 128:(k + 1) * 128, nsl])
                xtc.append(x_one)
            # qkT projection for this query chunk; pair-0 tiles first
            for mt in (0, 2, 4, 6, 1, 3, 5, 7):
                pp = ps.tile([128, 512], f32, name="accp", tag="acc", bufs=4)
                for k in range(KT):
                    nc.tensor.matmul(
                        pp,
                        wqk_sb[k][:, mt * 128:(mt + 1) * 128],
                        xtc[k],
                        start=(k == 0), stop=(k == KT - 1))
                nc.vector.tensor_scalar_add(qkt[mt][:, nsl], pp,
                                            bqk_sb[:, mt:mt + 1])
            # v projection for the 4 key tiles in this chunk
            for sl in range(4):
                st = nc_i * 4 + sl
                vp = ps.tile([128, HG * DH], f32, name="accv", tag="acc",
                             bufs=4)
                for k in range(KT):
                    nc.tensor.matmul(
                        vp,
                        xtc[k][:, sl * 128:(sl + 1) * 128],
                        wv_sb[k],
                        start=(k == 0), stop=(k == KT - 1))
                nc.vector.tensor_copy(
                    out=v_sb[:, st, :, 0:DH],
                    in_=vp.rearrange("p (h d) -> p h d", h=HG))

        projstack.close()

        # ---------------- attention ----------------
        e_pool = ctx.enter_context(tc.tile_pool(name="ep", bufs=3))
        oc_pool = ctx.enter_context(tc.tile_pool(name="oc", bufs=8))
        small = ctx.enter_context(tc.tile_pool(name="small", bufs=2))
        outst_pool = ctx.enter_context(tc.tile_pool(name="outst", bufs=3))
        scr_pool = ctx.enter_context(tc.tile_pool(name="scr", bufs=2,
                                                  space="DRAM"))

        for c in range(NCH):
            csl = slice(c * 512, (c + 1) * 512)
            ochl = [None] * HG    # oc tile per local head, this chunk
            for p in range(2):
                u_tiles = []
                for name in ("u1a", "u1b", "u2a", "u2b"):
                    u_tiles.append(ps.tile([DH + 1, 512], f32, name=name,
                                           tag="acc", bufs=4))
                for j in range(JT):
                    jsl = slice(j * 128, (j + 1) * 128)
                    s_ps = ps.tile([128, 2048], f32, name="s_ps", tag="s",
                                   bufs=1)
                    # scores transposed: sT[j, i]; eps = head parity in pair
                    for eps in range(2):
                        psl = slice(eps * 64, (eps + 1) * 64)
                        nc.tensor.matmul(
                            s_ps[:, eps * 512:(eps + 1) * 512],
                            qkt[4 + p][psl, jsl], qkt[0 + p][psl, csl],
                            start=True, stop=True)
                        nc.tensor.matmul(
                            s_ps[:, 1024 + eps * 512:1024 + (eps + 1) * 512],
                            qkt[6 + p][psl, jsl], qkt[2 + p][psl, csl],
                            start=True, stop=True)
                    e_sb = e_pool.tile([128, 2048], f32r, name="e_sb",
                                       tag="e")
                    nc.scalar.activation(e_sb, s_ps, Exp,
                                         bias=maskb_sb[:, j:j + 1],
                                         scale=SCALE)
                    for mi in range(2):        # 0 -> e1 block, 1 -> e2 block
                        for eps in range(2):
                            nc.tensor.matmul(
                                u_tiles[2 * mi + eps],
                                v_sb[:, j, 2 * p + eps, :],
                                e_sb[:, (2 * mi + eps) * 512:
                                        (2 * mi + eps + 1) * 512],
                                start=(j == 0), stop=(j == JT - 1))
                # normalization + combine:
                # o_hl = u1/dn1 - lam*u2/dn2 + (1-lam)*bv_hl
                for eps in range(2):
                    hl = 2 * p + eps
                    u1 = u_tiles[0 + eps]
                    u2 = u_tiles[2 + eps]
                    rb = small.tile([128, 1024], f32, name="rb", tag="rb",
                                    bufs=2)
                    nc.vector.reciprocal(rb[DH:DH + 1, 0:512],
                                         u1[DH:DH + 1, :])
                    nc.vector.reciprocal(rb[DH:DH + 1, 512:1024],
                                         u2[DH:DH + 1, :])
                    nc.vector.tensor_scalar_mul(rb[DH:DH + 1, 512:1024],
                                                rb[DH:DH + 1, 512:1024],
                                                float(lam))
                    # broadcast r over 64 partitions via a DRAM round-trip
                    # (SBUF-source partition-step-0 DMA is rejected)
                    scr = scr_pool.tile([1, 1024], f32, name="scr", tag="scr")
                    nc.sync.dma_start(out=scr, in_=rb[DH:DH + 1, :])
                    bc = small.tile([64, 1024], f32, name="bc", tag="bc",
                                    bufs=2)
                    nc.gpsimd.dma_start(
                        out=bc,
                        in_=scr.partition_broadcast(64)[:, 0, :])
                    t1 = small.tile([64, 512], f32, name="t1", tag="t1",
                                    bufs=2)
                    nc.vector.tensor_tensor(t1, u1[0:DH, :], bc[:, 0:512],
                                            mult)
                    t2 = small.tile([64, 512], f32, name="t2", tag="t2",
                                    bufs=2)
                    nc.vector.tensor_tensor(t2, u2[0:DH, :], bc[:, 512:1024],
                                            mult)
                    oc_t = oc_pool.tile([64, 512], f32r, name="oc_t",
                                        tag="oc")
                    # oc = (t1 + bvc_hl) - t2
                    nc.vector.scalar_tensor_tensor(
                        out=oc_t, in0=t1, scalar=bvc_sb[:, hl:hl + 1],
                        in1=t2, op0=add, op1=subtract)
                    ochl[hl] = oc_t

            # out projection for this query chunk: outT[:, csl] += Wo.T @ oT
            for mt in range(MT):
                op = ps.tile([128, 512], f32, name="accop", tag="acc", bufs=4)
                for hl in range(HG):
                    nc.tensor.matmul(op,
                                     wo_sb[hl][:, mt * 128:(mt + 1) * 128],
                                     ochl[hl],
                                     start=(hl == 0), stop=(hl == HG - 1))
                outst = outst_pool.tile([128, 512], f32, name="outst",
                                        tag="outst")
                nc.vector.tensor_copy(out=outst, in_=op)
                nc.sync.dma_start(out=out_d[mt * 128:(mt + 1) * 128, csl],
                                  in_=outst)

    nc.compile()
    return nc


def _get_nc(lam: float):
    key = round(float(lam), 8)
    if key not in _BUILD_CACHE:
        _BUILD_CACHE[key] = _build(float(lam))
    return _BUILD_CACHE[key]


def _prep_in_maps(hidden_states, attention_mask, Wq, bq, Wk, bk, Wv, bv, Wo,
                  lam_f):
    in_maps = []
    for d in range(NCORES):
        b, g = d // 4, d % 4
        gc = slice(g * HG * DH, (g + 1) * HG * DH)   # 256 head-group columns
        xt = np.ascontiguousarray(hidden_states[b].T)
        wqk = np.ascontiguousarray(
            np.concatenate([Wq[:, :D][:, gc], Wq[:, D:][:, gc],
                            Wk[:, :D][:, gc], Wk[:, D:][:, gc]], axis=1))
        wv = np.ascontiguousarray(Wv[:, gc])
        wo = np.ascontiguousarray(Wo[gc, :])
        bqk = np.ascontiguousarray(
            np.concatenate([bq[:D][gc], bq[D:][gc], bk[:D][gc], bk[D:][gc]])
            .reshape(MT, 128).T)
        bvc = np.ascontiguousarray(
            ((1.0 - lam_f) * bv[gc]).reshape(HG, 64).T)
        maskb = np.ascontiguousarray(
            ((1.0 - attention_mask[b]) * -10000.0).reshape(JT, 128).T)
        in_maps.append({"xt": xt, "wqk": wqk, "wv": wv, "wo": wo,
                        "bqk": bqk, "bvc": bvc, "maskb": maskb})
    return in_maps


def kernel(hidden_states, attention_mask, Wq, bq, Wk, bk, Wv, bv, Wo, bo,
           lam):
    hidden_states = np.asarray(hidden_states, dtype=np.float32)
    attention_mask = np.asarray(attention_mask, dtype=np.float32)
    Wq = np.asarray(Wq, dtype=np.float32)
    bq = np.asarray(bq, dtype=np.float32)
    Wk = np.asarray(Wk, dtype=np.float32)
    bk = np.asarray(bk, dtype=np.float32)
    Wv = np.asarray(Wv, dtype=np.float32)
    bv = np.asarray(bv, dtype=np.float32)
    Wo = np.asarray(Wo, dtype=np.float32)
    bo = np.asarray(bo, dtype=np.float32)
    lam_f = float(np.asarray(lam))

    from concourse.bass_utils import run_bass_kernel_spmd

    nc = _get_nc(lam_f)
    in_maps = _prep_in_maps(hidden_states, attention_mask, Wq, bq, Wk, bk,
                            Wv, bv, Wo, lam_f)
    res = run_bass_kernel_spmd(nc, in_maps, core_ids=list(range(NCORES)))

    out = np.zeros((B, S, D), np.float32)
    for d in range(NCORES):
        out[d // 4] += res.results[d]["outT"].T
    out += bo
    return out
